# revision 4
# baseline (speedup 1.0000x reference)
"""Trainium2 Bass kernel for a dense transformer block.

Data-parallel over batch B=8 across 8 NeuronCores (one batch element per
core, weights replicated, no collectives).

Per core (x_b is [T=1024, C=1024] fp32):
  h  = LN1(x);  per-head q,k,v = h @ Wq/Wk/Wv;  S = q k^T / 8 with the
  "staircase" mask, which is exactly block-causal at 64 granularity
  (row r attends to keys [0, (r//64+1)*64) );  out = softmax(S) v
  x2 = x + cat(out) @ Wo + bo;  y = x2 + relu(LN2(x2) @ W1 + b1) @ W2 + b2

Layout strategy:
  - token-major tiles [128 tokens, C] for LN / residuals / softmax rowsums
  - channel-major activations (transposed on the PE) feed every matmul
    contraction (K on partitions)
  - attention computes S^T [keys, queries] per head so exp(S^T) tiles are
    directly the stationary operand of the A@V matmul; a ones column
    appended to V yields the softmax denominator for free, and the
    denominator lands token-major where tensor_scalar can divide it out
"""

import os

import numpy as np

import concourse.bass as bass
import concourse.mybir as mybir
import concourse.tile as tile
from concourse import bacc
from concourse.masks import make_identity
from concourse.bass_utils import run_bass_kernel_spmd

T, C, H, HS = 1024, 1024, 16, 64
NT = T // 128          # 8 token tiles
NCH = C // 128         # 8 channel chunks
NPAIR = H // 2         # 8 head pairs
FF = 4 * C             # 4096
NG = FF // 128         # 32 FFN hidden groups
EPS = 1e-5
F32 = mybir.dt.float32

# Matmul input dtype view. float32 = exact, 4 cycles/row on the PE.
# float32r = TF32-like reduced precision, 1 cycle/row for N>=256.
MM_DT = getattr(mybir.dt, os.environ.get("KERNEL_MM_DT", "float32"))
TR_DT = getattr(mybir.dt, os.environ.get("KERNEL_TR_DT", "float32"))


def _mm(ap):
    return ap if MM_DT is F32 else ap.bitcast(MM_DT)


def _tr(ap):
    return ap if TR_DT is F32 else ap.bitcast(TR_DT)


def _layernorm(nc, pool, x_ap, out_ap, eps_tile, g_rep, b_rep):
    """LN along the free dim (C=1024) of a [128, 1024] tile."""
    stats = pool.tile([128, 2, 6], F32, tag="ln_stats", name="ln_stats")
    mv = pool.tile([128, 2], F32, tag="ln_mv", name="ln_mv")
    xr = x_ap.rearrange("p (s f) -> p s f", s=2)
    for s in range(2):
        nc.vector.bn_stats(out=stats[:, s, :], in_=xr[:, s, :])
    nc.vector.bn_aggr(out=mv, in_=stats)
    rstd = pool.tile([128, 1], F32, tag="ln_rstd", name="ln_rstd")
    nc.scalar.activation(
        out=rstd, in_=mv[:, 1:2],
        func=mybir.ActivationFunctionType.Sqrt,
        bias=eps_tile, scale=1.0,
    )
    nc.vector.reciprocal(out=rstd, in_=rstd)
    nc.vector.tensor_scalar(
        out=out_ap, in0=x_ap,
        scalar1=mv[:, 0:1], scalar2=rstd,
        op0=mybir.AluOpType.subtract, op1=mybir.AluOpType.mult,
    )
    nc.vector.tensor_mul(out=out_ap, in0=out_ap, in1=g_rep)
    nc.vector.tensor_add(out=out_ap, in0=out_ap, in1=b_rep)


def build_program():
    nc = bacc.Bacc("TRN2", target_bir_lowering=False, debug=False, num_devices=8)

    x_d = nc.dram_tensor("x", [T, C], F32, kind="ExternalInput").ap()
    wq_d = nc.dram_tensor("wq", [C, C], F32, kind="ExternalInput").ap()
    wk_d = nc.dram_tensor("wk", [C, C], F32, kind="ExternalInput").ap()
    wv_d = nc.dram_tensor("wv", [C, C], F32, kind="ExternalInput").ap()
    wo_d = nc.dram_tensor("wo", [C, C], F32, kind="ExternalInput").ap()
    w1_d = nc.dram_tensor("w1", [C, FF], F32, kind="ExternalInput").ap()
    w2_d = nc.dram_tensor("w2", [FF, C], F32, kind="ExternalInput").ap()
    bo_d = nc.dram_tensor("bo", [C], F32, kind="ExternalInput").ap()
    b1_d = nc.dram_tensor("b1", [FF], F32, kind="ExternalInput").ap()
    b2_d = nc.dram_tensor("b2", [C], F32, kind="ExternalInput").ap()
    ln1g_d = nc.dram_tensor("ln1g", [C], F32, kind="ExternalInput").ap()
    ln1b_d = nc.dram_tensor("ln1b", [C], F32, kind="ExternalInput").ap()
    ln2g_d = nc.dram_tensor("ln2g", [C], F32, kind="ExternalInput").ap()
    ln2b_d = nc.dram_tensor("ln2b", [C], F32, kind="ExternalInput").ap()
    y_d = nc.dram_tensor("y", [T, C], F32, kind="ExternalOutput").ap()

    with tile.TileContext(nc) as tc:
        _emit(nc, tc, x_d, wq_d, wk_d, wv_d, wo_d, w1_d, w2_d,
              bo_d, b1_d, b2_d, ln1g_d, ln1b_d, ln2g_d, ln2b_d, y_d)
    nc.compile()
    return nc


def _emit(nc, tc, x_d, wq_d, wk_d, wv_d, wo_d, w1_d, w2_d,
          bo_d, b1_d, b2_d, ln1g_d, ln1b_d, ln2g_d, ln2b_d, y_d):
    singles = tc.alloc_tile_pool(name="singles", bufs=1)
    ident = singles.tile([128, 128], F32, name="ident")
    make_identity(nc, ident)
    eps_tile = singles.tile([128, 1], F32, name="eps")
    nc.vector.memset(eps_tile, EPS)

    def rep(name, src):  # replicate a [C] vector across 128 partitions
        t = singles.tile([128, C], F32, tag=name, name=name)
        nc.gpsimd.dma_start(out=t, in_=src.unsqueeze(0).to_broadcast((128, C)))
        return t

    ln1g_r, ln1b_r = rep("ln1g_r", ln1g_d), rep("ln1b_r", ln1b_d)
    ln2g_r, ln2b_r = rep("ln2g_r", ln2g_d), rep("ln2b_r", ln2b_d)
    bo_r, b2_r = rep("bo_r", bo_d), rep("b2_r", b2_d)
    b1_sb = singles.tile([128, NG], F32, name="b1_sb")
    nc.sync.dma_start(out=b1_sb, in_=b1_d.rearrange("(g p) -> p g", p=128))

    ln_pool = tc.alloc_tile_pool(name="ln", bufs=3)

    # ---- Phase 1: LN1 + transpose to channel-major ----
    hT_pool = tc.alloc_tile_pool(name="hTp", bufs=1)
    hT = hT_pool.tile([128, NCH, T], F32, name="hT")
    with tc.tile_pool(name="h", bufs=2) as h_pool, \
         tc.tile_pool(name="xin1", bufs=2) as x_pool, \
         tc.tile_pool(name="trps", bufs=2, space="PSUM") as trps:
        for i in range(NT):
            x_t = x_pool.tile([128, C], F32, tag="x", name="x_t")
            nc.sync.dma_start(out=x_t, in_=x_d[i * 128:(i + 1) * 128, :])
            h_t = h_pool.tile([128, C], F32, tag="h", name="h_t")
            _layernorm(nc, ln_pool, x_t, h_t, eps_tile, ln1g_r, ln1b_r)
            for j in range(NCH):
                ps = trps.tile([128, 128], F32, tag="tr", name="ps_tr")
                nc.tensor.transpose(ps, _tr(h_t[:, j * 128:(j + 1) * 128]), _tr(ident))
                nc.vector.tensor_copy(out=hT[:, j, i * 128:(i + 1) * 128], in_=ps)

    # ---- Phase 2: per head-pair QKV + attention ----
    outT_pool = tc.alloc_tile_pool(name="outTp", bufs=1, side="right")
    outT = outT_pool.tile([128, NPAIR, T], F32, name="outT")

    with tc.tile_pool(name="wqkv", bufs=2) as w_pool, \
         tc.tile_pool(name="qk", bufs=2) as qk_pool, \
         tc.tile_pool(name="vp", bufs=2) as v_pool, \
         tc.tile_pool(name="expS", bufs=8) as e_pool, \
         tc.tile_pool(name="opair", bufs=2) as o_pool, \
         tc.tile_pool(name="rec", bufs=4) as r_pool, \
         tc.tile_pool(name="attps", bufs=1, space="PSUM") as aps:
        for p in range(NPAIR):
            # -- QKV for this pair (heads 2p, 2p+1) --
            wq_t = w_pool.tile([128, NCH, 128], F32, tag="wq", name="wq_t")
            wk_t = w_pool.tile([128, NCH, 128], F32, tag="wk", name="wk_t")
            wv_t = w_pool.tile([128, NCH, 128], F32, tag="wv", name="wv_t")
            csl = slice(p * 128, (p + 1) * 128)
            for wt, wd in ((wq_t, wq_d), (wk_t, wk_d), (wv_t, wv_d)):
                nc.sync.dma_start(
                    out=wt, in_=wd[:, csl].rearrange("(ch cp) n -> cp ch n", cp=128))

            qT = qk_pool.tile([128, T], F32, tag="qT", name="qT")   # [2*HS, T]
            kT = qk_pool.tile([128, T], F32, tag="kT", name="kT")
            for dst, wt in ((qT, wq_t), (kT, wk_t)):
                for half in range(2):
                    ps = aps.tile([128, 512], F32, tag="qkv", bufs=2, name="ps_qk")
                    for j in range(NCH):
                        nc.tensor.matmul(
                            ps, _mm(wt[:, j, :]),
                            _mm(hT[:, j, half * 512:(half + 1) * 512]),
                            start=(j == 0), stop=(j == NCH - 1))
                    nc.vector.tensor_copy(
                        out=dst[:, half * 512:(half + 1) * 512], in_=ps)

            # v token-major with a ones column per head: cols [65h, 65h+64]
            v_t = v_pool.tile([128, NT, 130], F32, tag="v", name="v_t")
            for hh in range(2):
                nc.gpsimd.memset(v_t[:, :, 65 * hh + 64:65 * hh + 65], 1.0)
            for i in range(NT):
                ps = aps.tile([128, 512], F32, tag="qkv", bufs=2, name="ps_v")
                for j in range(NCH):
                    nc.tensor.matmul(
                        ps[:, 0:128], _mm(hT[:, j, i * 128:(i + 1) * 128]),
                        _mm(wv_t[:, j, :]),
                        start=(j == 0), stop=(j == NCH - 1))
                nc.vector.tensor_copy(out=v_t[:, i, 0:64], in_=ps[:, 0:64])
                nc.vector.tensor_copy(out=v_t[:, i, 65:129], in_=ps[:, 64:128])

            # -- attention, in t-halves to bound expS residency --
            for th in range(2):
                t0 = th * 512
                njt = (th + 1) * 4          # s-tiles 0..njt-1 participate
                eS = [[None] * njt for _ in range(2)]
                for j in range(njt):
                    c0 = max(0, j * 128 - t0)   # first valid col in this half
                    for hh in range(2):
                        hsl = slice(hh * 64, (hh + 1) * 64)
                        ps = aps.tile([128, 512], F32, tag=f"sc{hh}", bufs=2,
                                      name="ps_sc")
                        nc.tensor.matmul(
                            ps[:, c0:512],
                            _mm(kT[hsl, j * 128:(j + 1) * 128]),
                            _mm(qT[hsl, t0 + c0:t0 + 512]),
                            start=True, stop=True,
                            tile_position=(hh * 64, 0))
                        et = e_pool.tile([128, 512], F32, tag=f"e{hh}", name="eS_t")
                        nc.scalar.activation(
                            out=et[:, c0:512], in_=ps[:, c0:512],
                            func=mybir.ActivationFunctionType.Exp,
                            scale=float(HS) ** -0.5)
                        if j * 128 >= t0:   # diagonal tile: zero masked quadrant
                            nc.gpsimd.memset(et[64:128, c0:c0 + 64], 0.0)
                        eS[hh][j] = et
                for it in range(th * 4, (th + 1) * 4):
                    ps_av = aps.tile([128, 130], F32, tag="av", name="ps_av")
                    for hh in range(2):
                        for j in range(it + 1):
                            nc.tensor.matmul(
                                ps_av[:, 65 * hh:65 * hh + 65],
                                _mm(eS[hh][j][:, it * 128 - t0:it * 128 - t0 + 128]),
                                _mm(v_t[:, j, 65 * hh:65 * hh + 65]),
                                start=(j == 0), stop=(j == it))
                    o_t = o_pool.tile([128, 128], F32, tag="o", name="o_t")
                    for hh in range(2):
                        rc = r_pool.tile([128, 1], F32, tag="r", name="rc")
                        nc.vector.reciprocal(
                            out=rc, in_=ps_av[:, 65 * hh + 64:65 * hh + 65])
                        nc.vector.tensor_scalar(
                            out=o_t[:, hh * 64:(hh + 1) * 64],
                            in0=ps_av[:, 65 * hh:65 * hh + 64],
                            scalar1=rc, scalar2=None,
                            op0=mybir.AluOpType.mult)
                    ps_tr = aps.tile([128, 128], F32, tag="tr", name="ps_otr")
                    nc.tensor.transpose(ps_tr, _tr(o_t), _tr(ident))
                    nc.vector.tensor_copy(
                        out=outT[:, p, it * 128:(it + 1) * 128], in_=ps_tr)
    hT_pool.release()

    # ---- Phase 3: output projection + residual ----
    x2_pool = tc.alloc_tile_pool(name="x2p", bufs=1)
    x2 = x2_pool.tile([128, NT, C], F32, name="x2")
    with tc.tile_pool(name="wo", bufs=1) as wo_pool, \
         tc.tile_pool(name="xin2", bufs=2) as x_pool, \
         tc.tile_pool(name="prps", bufs=2, space="PSUM") as prps:
        wo_t = wo_pool.tile([128, NCH, C], F32, name="wo_t")
        nc.sync.dma_start(out=wo_t, in_=wo_d.rearrange("(ch cp) n -> cp ch n", cp=128))
        for i in range(NT):
            x_t = x_pool.tile([128, C], F32, tag="x", name="x_t2")
            nc.sync.dma_start(out=x_t, in_=x_d[i * 128:(i + 1) * 128, :])
            for half in range(2):
                ps = prps.tile([128, 512], F32, tag="pr", name="ps_pr")
                for ch in range(NCH):
                    nc.tensor.matmul(
                        ps, _mm(outT[:, ch, i * 128:(i + 1) * 128]),
                        _mm(wo_t[:, ch, half * 512:(half + 1) * 512]),
                        start=(ch == 0), stop=(ch == NCH - 1))
                hsl = slice(half * 512, (half + 1) * 512)
                nc.vector.tensor_add(out=x2[:, i, hsl], in0=ps, in1=x_t[:, hsl])
                nc.vector.tensor_add(
                    out=x2[:, i, hsl], in0=x2[:, i, hsl], in1=bo_r[:, hsl])
    outT_pool.release()

    # ---- Phase 4: LN2 + transpose ----
    h2T_pool = tc.alloc_tile_pool(name="h2Tp", bufs=1, side="right")
    h2T = h2T_pool.tile([128, NCH, T], F32, name="h2T")
    with tc.tile_pool(name="h2", bufs=2) as h2_pool, \
         tc.tile_pool(name="trps2", bufs=2, space="PSUM") as trps2:
        for i in range(NT):
            h_t = h2_pool.tile([128, C], F32, tag="h2", name="h2_t")
            _layernorm(nc, ln_pool, x2[:, i, :], h_t, eps_tile, ln2g_r, ln2b_r)
            for j in range(NCH):
                ps = trps2.tile([128, 128], F32, tag="tr2", name="ps_tr2")
                nc.tensor.transpose(ps, _tr(h_t[:, j * 128:(j + 1) * 128]), _tr(ident))
                nc.vector.tensor_copy(out=h2T[:, j, i * 128:(i + 1) * 128], in_=ps)

    # ---- Phase 5: FFN in t-halves ----
    with tc.tile_pool(name="w1", bufs=3) as w1_pool, \
         tc.tile_pool(name="w2", bufs=2) as w2_pool, \
         tc.tile_pool(name="uTp", bufs=1) as uT_pool, \
         tc.tile_pool(name="yout", bufs=2) as out_pool:
        for th in range(2):
            t0 = th * 512
            uT = uT_pool.tile([128, NG, 512], F32, tag="uT", name="uT")
            with tc.tile_pool(name="ups", bufs=2, space="PSUM") as ups:
                for g in range(NG):
                    w1_t = w1_pool.tile([128, NCH, 128], F32, tag="w1", name="w1_t")
                    nc.sync.dma_start(
                        out=w1_t,
                        in_=w1_d[:, g * 128:(g + 1) * 128].rearrange(
                            "(ch cp) n -> cp ch n", cp=128))
                    ps = ups.tile([128, 512], F32, tag="u", name="ps_u")
                    for j in range(NCH):
                        nc.tensor.matmul(
                            ps, _mm(w1_t[:, j, :]),
                            _mm(h2T[:, j, t0:t0 + 512]),
                            start=(j == 0), stop=(j == NCH - 1))
                    nc.scalar.activation(
                        out=uT[:, g, :], in_=ps,
                        func=mybir.ActivationFunctionType.Relu,
                        bias=b1_sb[:, g:g + 1], scale=1.0)
            with tc.tile_pool(name="fps", bufs=1, space="PSUM") as fps:
                ps_f = [[fps.tile([128, 512], F32, tag=f"f{it}{chh}",
                                  name=f"ps_f{it}{chh}")
                         for chh in range(2)] for it in range(4)]
                for k in range(NG):
                    w2_t = w2_pool.tile([128, C], F32, tag="w2", name="w2_t")
                    nc.sync.dma_start(out=w2_t, in_=w2_d[k * 128:(k + 1) * 128, :])
                    for it in range(4):
                        for chh in range(2):
                            nc.tensor.matmul(
                                ps_f[it][chh],
                                _mm(uT[:, k, it * 128:(it + 1) * 128]),
                                _mm(w2_t[:, chh * 512:(chh + 1) * 512]),
                                start=(k == 0), stop=(k == NG - 1))
                for it in range(4):
                    gi = th * 4 + it
                    for chh in range(2):
                        hsl = slice(chh * 512, (chh + 1) * 512)
                        o_t = out_pool.tile([128, 512], F32, tag="y", name="y_t")
                        nc.vector.tensor_add(
                            out=o_t, in0=ps_f[it][chh], in1=x2[:, gi, hsl])
                        nc.vector.tensor_add(out=o_t, in0=o_t, in1=b2_r[:, hsl])
                        nc.sync.dma_start(
                            out=y_d[gi * 128:(gi + 1) * 128, hsl], in_=o_t)
    x2_pool.release()
    h2T_pool.release()
    ln_pool.release()
    singles.release()


_NC_CACHE = {}


def _get_program():
    if "nc" not in _NC_CACHE:
        _NC_CACHE["nc"] = build_program()
    return _NC_CACHE["nc"]


def _prep_inputs(x, Wq, Wk, Wv, Wo, bo, ln1_g, ln1_b, ln2_g, ln2_b, W1, b1, W2, b2):
    f = lambda a: np.ascontiguousarray(np.asarray(a, dtype=np.float32))
    wq2 = f(np.asarray(Wq, np.float32).transpose(1, 0, 2).reshape(C, C))
    wk2 = f(np.asarray(Wk, np.float32).transpose(1, 0, 2).reshape(C, C))
    wv2 = f(np.asarray(Wv, np.float32).transpose(1, 0, 2).reshape(C, C))
    return {
        "wq": wq2, "wk": wk2, "wv": wv2, "wo": f(Wo), "w1": f(W1), "w2": f(W2),
        "bo": f(bo), "b1": f(b1), "b2": f(b2),
        "ln1g": f(ln1_g), "ln1b": f(ln1_b), "ln2g": f(ln2_g), "ln2b": f(ln2_b),
    }


def kernel(x, mask, Wq, Wk, Wv, Wo, bo, ln1_g, ln1_b, ln2_g, ln2_b, W1, b1, W2, b2):
    x = np.ascontiguousarray(np.asarray(x, dtype=np.float32))
    B = x.shape[0]
    common = _prep_inputs(x, Wq, Wk, Wv, Wo, bo, ln1_g, ln1_b,
                          ln2_g, ln2_b, W1, b1, W2, b2)
    nc = _get_program()
    in_maps = [dict(common, x=np.ascontiguousarray(x[b])) for b in range(B)]
    res = run_bass_kernel_spmd(nc, in_maps, list(range(B)))
    return np.stack([res.results[b]["y"] for b in range(B)], axis=0)
